# revision 1
# baseline (speedup 1.0000x reference)
"""Causal self-attention (B=2, T=2048, C=1024, H=16) on 8 trn2 NeuronCores.

Sharding: core c = (b, g) with b = c // 4 (batch), g = c % 4 (head-group of 4
heads = 256 dims).  No collectives: each core computes a PARTIAL output
projection over its own 256 head-dims (o_part^T = W_g^T y_g^T, bf16) and the
host sums the 4 partials per batch during unsharding.

Per core, software-pipelined in 4 stages over 512-wide t-chunks:
  stage cq: attention chunk cq (flash-style S^T = K Q^T, exp on ACT, causal
  mask on GpSimd, AV with [V | 1] stationary) -> normalize (fast approx
  reciprocal on DVE + partition-broadcast on GpSimd) -> QKV projection for
  t-chunk cq+1 (so the PE never waits on the normalize chain) -> partial
  output projection of chunk cq -> DMA out.
QKV for t-chunk 0 plus a PE p-state warmup run in the prologue while the
weights/x DMAs stream in.
"""
import math

import numpy as np
import ml_dtypes

B, T, C, H = 2, 2048, 1024, 16
HD = C // H          # 64 head dim
G = 4                # head-groups (cores per batch)
HPG = H // G         # 4 heads per group
DG = HPG * HD        # 256 dims per group
N_CORES = 8
KC = C // 128        # 8 contraction chunks
NTC = T // 512       # 4 t-chunks (and attention q-chunks)
VW = HD + 2          # V1 per-head stride (64 data + 1 ones + 1 pad)

_NC_CACHE = {}


def _build():
    import concourse.bacc as bacc
    import concourse.mybir as mybir
    import concourse.tile as tile

    f32 = mybir.dt.float32
    f32r = mybir.dt.float32r
    bf16 = mybir.dt.bfloat16
    Exp = mybir.ActivationFunctionType.Exp
    Ident = mybir.ActivationFunctionType.Identity
    Copy = mybir.ActivationFunctionType.Copy

    nc = bacc.Bacc("TRN2", num_devices=N_CORES)

    xT_d = nc.dram_tensor("xT", [C, T], bf16, kind="ExternalInput")
    wq_d = nc.dram_tensor("wq", [C, DG], bf16, kind="ExternalInput")
    wk_d = nc.dram_tensor("wk", [C, DG], bf16, kind="ExternalInput")
    wv_d = nc.dram_tensor("wv", [C, DG], bf16, kind="ExternalInput")
    bq_d = nc.dram_tensor("bq", [2, 128, 1], f32, kind="ExternalInput")
    bk_d = nc.dram_tensor("bk", [2, 128, 1], f32, kind="ExternalInput")
    bv_d = nc.dram_tensor("bv", [1, DG], f32, kind="ExternalInput")
    # w_proj.T rows [lo:lo+DG] -> partial projection stationary [DG, C]
    wp_d = nc.dram_tensor("wpT", [DG, C], bf16, kind="ExternalInput")
    mask_d = nc.dram_tensor("mask", [128, 128], bf16, kind="ExternalInput")
    oP_d = nc.dram_tensor("oP", [C, T], bf16, kind="ExternalOutput")

    def dma_chunked(dst, src, n):
        w = dst.shape[-1]
        step = w // n
        for i in range(n):
            nc.sync.dma_start(dst[..., step * i:step * (i + 1)],
                              src[..., step * i:step * (i + 1)])

    with tile.TileContext(nc) as tc:
        with (
            tc.tile_pool(name="persist", bufs=1) as persist,
            tc.tile_pool(name="xp", bufs=1) as xp,
            tc.tile_pool(name="wp_s", bufs=1) as wp_s,
            tc.tile_pool(name="psp", bufs=1, space="PSUM") as psp,
            tc.tile_pool(name="ppool", bufs=1) as ppool,
            tc.tile_pool(name="npool", bufs=1) as npool,
            tc.tile_pool(name="ynp", bufs=1) as ynp,
            tc.tile_pool(name="otp", bufs=1) as otp,
        ):
            # ---- persistent SBUF ----
            QT = [[persist.tile([128, 512], f32r, name=f"qt{t}_{j}")
                   for j in range(2)] for t in range(NTC)]
            KT = [[persist.tile([128, 512], f32r, name=f"kt{t}_{j}")
                   for j in range(2)] for t in range(NTC)]
            V1 = [persist.tile([128, HPG * VW], bf16, name=f"v{m}")
                  for m in range(4 * NTC)]
            xT_sb = [[xp.tile([128, 512], bf16, name=f"x{t}_{k}")
                      for k in range(KC)] for t in range(NTC)]
            wq_sb = [wp_s.tile([128, DG], bf16, name=f"wq{k}") for k in range(KC)]
            wk_sb = [wp_s.tile([128, DG], bf16, name=f"wk{k}") for k in range(KC)]
            wv_sb = [wp_s.tile([128, DG], bf16, name=f"wv{k}") for k in range(KC)]
            wpT_sb = [persist.tile([128, C], bf16, name=f"wp_{k}")
                      for k in range(2)]
            mask_sb = persist.tile([128, 128], bf16, name="mask_sb")
            bq_sb = [persist.tile([128, 1], f32, name=f"bq{j}") for j in range(2)]
            bk_sb = [persist.tile([128, 1], f32, name=f"bk{j}") for j in range(2)]
            bv_row = persist.tile([1, DG], f32, name="bv_row")
            bv_bc = persist.tile([128, DG], f32, name="bv_bc")

            # ---- prologue: warmup + DMA streaming ----
            wu_a = wp_s.tile([128, 128], bf16, name="wu_a")
            wu_b = wp_s.tile([128, 512], bf16, name="wu_b")
            nc.vector.memset(wu_a[:], 0.5)
            nc.vector.memset(wu_b[:], 0.5)
            for i in range(16):
                wu_ps = psp.tile([128, 512], f32, tag="aux", bufs=2,
                                 name=f"wu{i}")
                nc.tensor.matmul(wu_ps[:], wu_a[:], wu_b[:],
                                 start=True, stop=True)

            # input DMA spread across per-engine HWDGE queues
            # (one queue runs at only ~75 GB/s; serialize = 85us of input)
            # gpsimd DMAs run on the software DGE and BLOCK the GpSimd
            # engine (starving masks/normalize) -- use only the sync and
            # scalar hardware-DGE queues, which are fire-and-forget.
            for k in range(KC):
                eng = nc.sync if k % 2 == 0 else nc.scalar
                eng.dma_start(xT_sb[0][k][:],
                              xT_d[128 * k:128 * (k + 1), 0:512])
                eng2 = nc.scalar if k % 2 == 0 else nc.sync
                eng2.dma_start(wq_sb[k][:], wq_d[128 * k:128 * (k + 1), :])
            for k in range(KC):
                nc.scalar.dma_start(wk_sb[k][:], wk_d[128 * k:128 * (k + 1), :])
            for k in range(KC):
                nc.scalar.dma_start(wv_sb[k][:], wv_d[128 * k:128 * (k + 1), :])
            for k in range(2):
                nc.scalar.dma_start(wpT_sb[k][:], wp_d[128 * k:128 * (k + 1), :])
            nc.scalar.dma_start(mask_sb[:], mask_d[:])
            for j in range(2):
                nc.scalar.dma_start(bq_sb[j][:], bq_d[j])
                nc.scalar.dma_start(bk_sb[j][:], bk_d[j])
            nc.scalar.dma_start(bv_row[:], bv_d[:])
            nc.gpsimd.partition_broadcast(bv_bc[:], bv_row[:])
            for t in range(1, NTC):
                for k in range(KC):
                    eng = nc.sync if k % 2 == 0 else nc.scalar
                    eng.dma_start(xT_sb[t][k][:],
                                  xT_d[128 * k:128 * (k + 1),
                                       512 * t:512 * (t + 1)])
            # ones columns of V1 (written once; disjoint from the data cols)
            for m in range(4 * NTC):
                vv = V1[m].rearrange("p (h x) -> p h x", h=HPG)
                nc.vector.memset(vv[:, :, HD:HD + 1], 1.0)

            def qkv(t):
                for sel, (wsb, dst, bcol) in enumerate((
                    (wq_sb, QT, bq_sb),
                    (wk_sb, KT, bk_sb),
                )):
                    for jh in range(2):
                        ps = psp.tile([128, 512], f32, tag="aux", bufs=2,
                                      name=f"qk{t}_{sel}_{jh}")
                        for kc in range(KC):
                            nc.tensor.matmul(
                                ps[:],
                                wsb[kc][:, 128 * jh:128 * (jh + 1)],
                                xT_sb[t][kc][:],
                                start=(kc == 0), stop=(kc == KC - 1))
                        if sel == 0:
                            nc.vector.tensor_scalar_add(dst[t][jh][:], ps[:],
                                                        bcol[jh][:])
                        else:
                            nc.scalar.activation(out=dst[t][jh][:], in_=ps[:],
                                                 func=Ident, bias=bcol[jh][:])
                for mt in range(4):
                    psv = psp.tile([128, 512], f32, tag="aux", bufs=2,
                                   name=f"vps{t}_{mt}")
                    for kc in range(KC):
                        nc.tensor.matmul(
                            psv[:, 0:DG],
                            xT_sb[t][kc][:, 128 * mt:128 * (mt + 1)],
                            wv_sb[kc][:],
                            start=(kc == 0), stop=(kc == KC - 1))
                    vv = V1[4 * t + mt].rearrange("p (h x) -> p h x", h=HPG)
                    nc.vector.tensor_add(
                        vv[:, :, 0:HD],
                        psv[:, 0:DG].rearrange("p (h x) -> p h x", h=HPG),
                        bv_bc.rearrange("p (h x) -> p h x", h=HPG))

            def att(cq):
                nkt = 4 * (cq + 1)
                yns = []
                for p in range(2):
                    yps = [psp.tile([HD + 1, 512], f32, tag=f"y{X}", bufs=1,
                                    name=f"y_{cq}_{p}_{X}") for X in range(2)]

                    def emit_av(kt, Pt, qs):
                        for X in range(2):
                            h = 2 * p + X
                            nc.tensor.matmul(
                                yps[X][:, qs:512],
                                V1[kt][:, VW * h:VW * h + HD + 1],
                                Pt[:, 512 * X + qs:512 * (X + 1)],
                                start=(kt == 0), stop=(kt == nkt - 1))

                    pend = None   # AV runs one k-tile behind S/exp
                    for kt in range(nkt):
                        qs = max(0, 128 * kt - 512 * cq)
                        qs2 = min(qs, 256)   # keep f32r free dim >= 256
                        S = psp.tile([128, 1024], f32, tag="s", bufs=2,
                                     name=f"s_{cq}_{p}_{kt}")
                        for X in range(2):
                            nc.tensor.matmul(
                                S[:, 512 * X + qs2:512 * (X + 1)],
                                KT[kt // 4][p][64 * X:64 * (X + 1),
                                               128 * (kt % 4):128 * (kt % 4 + 1)],
                                QT[cq][p][64 * X:64 * (X + 1), qs2:512],
                                start=True, stop=True)
                        if pend is not None:
                            emit_av(*pend)
                        Pt = ppool.tile([128, 1024], bf16, tag="p", bufs=4,
                                        name=f"p_{cq}_{p}_{kt}")
                        nc.scalar.activation(
                            out=Pt.rearrange("pp (x q) -> pp x q",
                                             x=2)[:, :, qs:512],
                            in_=S.rearrange("pp (x q) -> pp x q",
                                            x=2)[:, :, qs:512],
                            func=Exp, scale=1.0 / math.sqrt(HD))
                        if kt >= 4 * cq:  # diagonal block: causal mask
                            for X in range(2):
                                nc.gpsimd.tensor_mul(
                                    Pt[:, 512 * X + qs:512 * X + qs + 128],
                                    Pt[:, 512 * X + qs:512 * X + qs + 128],
                                    mask_sb[:])
                        pend = (kt, Pt, qs)
                    emit_av(*pend)
                    # normalize: drain psum fast, then recip/broadcast/mul
                    yn = ynp.tile([128, 512], bf16, tag="yn", bufs=4,
                                  name=f"yn_{cq}_{p}")
                    for X in range(2):
                        ycp = npool.tile([HD, 512], bf16, tag="ycp", bufs=4,
                                         name=f"yc_{cq}_{p}_{X}")
                        nc.vector.tensor_copy(ycp[:], yps[X][0:HD, :])
                        r1 = npool.tile([1, 512], f32, tag="r1", bufs=4,
                                        name=f"r1_{cq}_{p}_{X}")
                        nc.vector.tensor_copy(r1[:], yps[X][HD:HD + 1, :])
                        rr = npool.tile([1, 512], f32, tag="rr", bufs=4,
                                        name=f"rr_{cq}_{p}_{X}")
                        nc.vector.reciprocal_approx_fast(out=rr[:], in_=r1[:])
                        rrb = npool.tile([1, 512], bf16, tag="rrb", bufs=4,
                                         name=f"rrb_{cq}_{p}_{X}")
                        nc.vector.tensor_copy(rrb[:], rr[:])
                        bcx = npool.tile([HD, 512], bf16, tag="bc", bufs=4,
                                         name=f"bcx_{cq}_{p}_{X}")
                        nc.gpsimd.partition_broadcast(bcx[:], rrb[:])
                        nc.gpsimd.tensor_mul(
                            yn[64 * X:64 * (X + 1), :], ycp[:], bcx[:])
                    yns.append(yn)
                return yns

            def proj(cq, yns):
                for eh in range(KC):
                    po = psp.tile([128, 512], f32, tag="aux", bufs=2,
                                  name=f"po_{cq}_{eh}")
                    nc.tensor.matmul(po[:],
                                     wpT_sb[0][:, 128 * eh:128 * (eh + 1)],
                                     yns[0][:], start=True, stop=False)
                    nc.tensor.matmul(po[:],
                                     wpT_sb[1][:, 128 * eh:128 * (eh + 1)],
                                     yns[1][:], start=False, stop=True)
                    ot = otp.tile([128, 512], bf16, tag="ot", bufs=4,
                                  name=f"ot_{cq}_{eh}")
                    if eh % 2 == 0:
                        nc.vector.tensor_copy(ot[:], po[:])
                    else:
                        nc.scalar.activation(out=ot[:], in_=po[:], func=Copy)
                    nc.sync.dma_start(
                        oP_d[128 * eh:128 * (eh + 1),
                             512 * cq:512 * (cq + 1)], ot[:])

            # proj(cq) runs one stage late so the in-order PE queue never
            # waits on the normalize chain that produces yn
            qkv(0)
            prev = None
            for cq in range(NTC):
                yns = att(cq)
                if cq + 1 < NTC:
                    qkv(cq + 1)
                if prev is not None:
                    proj(cq - 1, prev)
                prev = yns
            proj(NTC - 1, prev)

    nc.finalize()
    return nc


def _get_nc():
    if "nc" not in _NC_CACHE:
        _NC_CACHE["nc"] = _build()
    return _NC_CACHE["nc"]


def kernel(x, w_attn, b_attn, w_proj, b_proj):
    from concourse.bass_utils import run_bass_kernel_spmd

    x = np.asarray(x, dtype=np.float32)
    w_attn = np.asarray(w_attn, dtype=np.float32)
    b_attn = np.asarray(b_attn, dtype=np.float32)
    w_proj = np.asarray(w_proj, dtype=np.float32)
    b_proj = np.asarray(b_proj, dtype=np.float32)

    mask = np.triu(np.ones((128, 128), dtype=np.float32)).copy()
    wpT_full = np.ascontiguousarray(w_proj.T)  # [C_in, C_out]

    in_maps = []
    for c in range(N_CORES):
        b, g = divmod(c, G)
        lo = DG * g
        in_maps.append({
            "xT": np.ascontiguousarray(x[b].T).astype(ml_dtypes.bfloat16),
            "wq": np.ascontiguousarray(w_attn[lo:lo + DG, :].T).astype(ml_dtypes.bfloat16),
            "wk": np.ascontiguousarray(w_attn[C + lo:C + lo + DG, :].T).astype(ml_dtypes.bfloat16),
            "wv": np.ascontiguousarray(w_attn[2 * C + lo:2 * C + lo + DG, :].T).astype(ml_dtypes.bfloat16),
            "bq": np.ascontiguousarray(b_attn[lo:lo + DG].reshape(2, 128, 1)),
            "bk": np.ascontiguousarray(
                b_attn[C + lo:C + lo + DG].reshape(2, 128, 1)),
            "bv": np.ascontiguousarray(
                b_attn[2 * C + lo:2 * C + lo + DG].reshape(1, DG)),
            "wpT": np.ascontiguousarray(wpT_full[lo:lo + DG, :]).astype(ml_dtypes.bfloat16),
            "mask": mask.astype(ml_dtypes.bfloat16),
        })

    global _last_in_maps
    _last_in_maps = in_maps

    nc = _get_nc()
    res = run_bass_kernel_spmd(nc, in_maps, list(range(N_CORES)))

    out = np.empty((B, T, C), dtype=np.float32)
    for b in range(B):
        acc = np.zeros((C, T), dtype=np.float32)
        for g in range(G):
            acc += res.results[4 * b + g]["oP"].astype(np.float32)
        out[b] = acc.T + b_proj
    return out

